# revision 1
# baseline (speedup 1.0000x reference)
"""Multi-head attention (B=4, L=1024, D=1024, H=16) on 8 TRN2 NeuronCores.

Sharding: pure data-parallel over (batch, query-half) — core c handles batch
c//2, query rows [512*(c%2), 512*(c%2+1)). Each core computes Q/K/V
projections for its batch (K/V duplicated across the 2 cores of a batch),
full attention for its 512 queries, and the output projection for its slice.
No collectives; the host concatenates the 8 output slices.

Everything on-device is kept in transposed layouts so no transposes are
needed anywhere:
  Q^T[vd, q]  = Wq(lhsT) @ qT(rhs)         (+bq per-partition via DVE)
  K^T[vd, k]  = Wk(lhsT) @ xT(rhs)         (+bk per-partition)
  V  [k, vd]  = xT(lhsT) @ Wv(rhs)         (+bv via K=1 ones-row matmul)
  S^T[k, q]   = K^T_h(lhsT, K=64) @ Q^T_h  for a head PAIR into one
                [128, 2, 512] 2-bank PSUM tile (heads 2j / 2j+1 at the two
                PE row groups), one exp per k-tile
  expS        = exp(S^T/8 + kmask_bias)    (ScalarE, PSUM->SBUF bf16)
  O^T+denom   = V_aug(lhsT, M=65) @ expS   (V cols + ones col per head)
  scale       = 1/denom broadcast to 64 partitions via K=1 ones matmul
  out[q, d]   = (O^T_scaled.T @ Wo) * q_mask + bo   (DVE epilogue)
"""

import os

os.environ.setdefault("MYCRO_LOCAL_CACHE", "1")

import numpy as np
import ml_dtypes

BF16 = ml_dtypes.bfloat16

B, LQ, LK = 4, 1024, 1024
D = 1024  # QD = KD = VD
H, DH = 16, 64
QS = 512  # queries per core
NCORES = 8
NEG = -1e4  # additive key-mask bias (exp(-1e4) == 0 in f32)

_NC_CACHE = {}


def _build_nc():
    import concourse.bacc as bacc
    import concourse.mybir as mybir
    import concourse.tile as tile

    dt = mybir.dt

    nc = bacc.Bacc(
        "TRN2",
        debug=False,
        target_bir_lowering=False,
        num_devices=NCORES,
    )

    def din(name, shape, dtype):
        return nc.dram_tensor(name, shape, dtype, kind="ExternalInput").ap()

    aps = {
        "qT": din("qT", [D, QS], dt.bfloat16),
        "xT": din("xT", [D, LK], dt.bfloat16),
        "Wq": din("Wq", [D, D], dt.bfloat16),
        "Wk": din("Wk", [D, D], dt.bfloat16),
        "Wv": din("Wv", [D, D], dt.bfloat16),
        "Wo": din("Wo", [D, D], dt.bfloat16),
        # packed per-partition constants: cols 0-7 bq, 8-15 bk, 16-23 kbias,
        # 24-27 q_mask (by query tile)
        "consts": din("consts", [128, 28], dt.float32),
        "bvr": din("bvr", [1, D], dt.bfloat16),
        "bor": din("bor", [1, D], dt.bfloat16),
        "out": nc.dram_tensor("out", [QS, D], dt.float32,
                              kind="ExternalOutput").ap(),
    }

    with tile.TileContext(nc) as tc:
        _body(tc, dt, mybir, aps)

    nc.compile()
    return nc


def _body(tc, dt, mybir, aps):
    from contextlib import ExitStack

    ALU = mybir.AluOpType
    AF = mybir.ActivationFunctionType
    nc = tc.nc
    with ExitStack() as ctx:
        const = ctx.enter_context(tc.tile_pool(name="const", bufs=1))
        espool = ctx.enter_context(tc.tile_pool(name="es", bufs=12))
        psum = ctx.enter_context(tc.tile_pool(name="psum", bufs=4, space="PSUM"))
        spair = ctx.enter_context(tc.tile_pool(name="spair", bufs=2, space="PSUM"))
        opool = ctx.enter_context(tc.tile_pool(name="osb", bufs=3))
        srpool = ctx.enter_context(tc.tile_pool(name="srp", bufs=2))

        def ctile(shape, dtype, tag):
            return const.tile(shape, dtype, tag=tag, name=tag)

        # ---- inputs: halved big strided DMAs, ordered by first use ----
        cst = ctile([128, 28], dt.float32, "cst")
        nc.sync.dma_start(cst[:], aps["consts"][:, :])
        bv_sb = ctile([1, D], dt.bfloat16, "bv")
        nc.scalar.dma_start(bv_sb[:], aps["bvr"][:, :])
        bo_sb = ctile([1, D], dt.bfloat16, "bo")
        nc.scalar.dma_start(bo_sb[:], aps["bor"][:, :])

        halves = {}
        engs = [nc.sync, nc.scalar, nc.gpsimd]
        ei = [0]

        def load_chunked(name, dram, nfree, nchunks):
            per = 8 // nchunks
            tiles = []
            view = dram.rearrange("(t p) n -> p t n", p=128)
            for ch in range(nchunks):
                tl = ctile([128, per, nfree], dt.bfloat16, f"{name}_{ch}")
                engs[ei[0] % 3].dma_start(
                    tl[:, :, :], view[:, per * ch:per * (ch + 1), :])
                ei[0] += 1
                tiles.append(tl)
            halves[name] = (tiles, per)

        def tile_of(name, t):
            tiles, per = halves[name]
            return tiles[t // per][:, t % per, :]

        load_chunked("qT", aps["qT"], QS, 2)
        load_chunked("wq", aps["Wq"], D, 4)
        load_chunked("xT", aps["xT"], LK, 4)
        load_chunked("wk", aps["Wk"], D, 4)
        load_chunked("wv", aps["Wv"], D, 2)
        load_chunked("wo", aps["Wo"], D, 2)

        bq_c = lambda j: cst[:, j:j + 1]
        bk_c = lambda j: cst[:, 8 + j:9 + j]
        kb_c = lambda kt: cst[:, 16 + kt:17 + kt]
        qm_c = lambda qt: cst[:, 24 + qt:25 + qt]

        ones1 = ctile([1, 128], dt.bfloat16, "ones1")
        nc.gpsimd.memset(ones1[:], 1.0)
        ones64 = ctile([1, 64], dt.bfloat16, "ones64")
        nc.gpsimd.memset(ones64[:], 1.0)

        # bo broadcast to all partitions (final tiles add it with DVE)
        bo_rep = ctile([128, D], dt.float32, "bo_rep")
        for n in range(2):
            c = slice(512 * n, 512 * (n + 1))
            ps = psum.tile([128, 512], dt.float32, tag="ps", name="ps")
            nc.tensor.matmul(ps[:], ones1[:], bo_sb[:, c], start=True, stop=True)
            nc.vector.tensor_copy(bo_rep[:, c], ps[:])

        # ---- Q^T projection (first: needs only qT+Wq, ~3 MB) ----
        qTp = [ctile([128, QS], dt.bfloat16, f"qTp{j}") for j in range(8)]
        for j in range(8):
            ps = psum.tile([128, QS], dt.float32, tag="ps", name="ps")
            for kt in range(8):
                nc.tensor.matmul(
                    ps[:], tile_of("wq", kt)[:, 128 * j:128 * (j + 1)],
                    tile_of("qT", kt)[:], start=(kt == 0), stop=(kt == 7))
            nc.vector.tensor_scalar_add(qTp[j][:], ps[:], bq_c(j))

        # ---- K^T projection, vd-tile j = heads (2j, 2j+1) ----
        kT_sb = [ctile([128, LK], dt.bfloat16, f"kT{j}") for j in range(8)]

        def k_proj(j):
            for n in range(2):
                c = slice(512 * n, 512 * (n + 1))
                ps = psum.tile([128, 512], dt.float32, tag="ps", name="ps")
                for kt in range(8):
                    nc.tensor.matmul(
                        ps[:], tile_of("wk", kt)[:, 128 * j:128 * (j + 1)],
                        tile_of("xT", kt)[:, c], start=(kt == 0), stop=(kt == 7))
                nc.vector.tensor_scalar_add(kT_sb[j][:, c], ps[:], bk_c(j))

        k_proj(0)
        k_proj(1)

        # ---- V projection into V_aug layout: per k-tile [128, 16*(64+1)],
        # head h at cols [65h, 65h+64), ones at col 65h+64. Tiles 3-7 are
        # traced INSIDE pair 0's attention stream (3 steps before their
        # o_stage consumer) so the exp pipeline starts ~40us earlier instead
        # of waiting behind the whole V projection in the in-order PE queue.
        v_sb = [ctile([128, H * (DH + 1)], dt.bfloat16, f"v{t}") for t in range(8)]
        for t in range(8):
            ones_cols = v_sb[t][:].rearrange(
                "p (h c) -> p h c", c=DH + 1)[:, :, DH:DH + 1]
            nc.gpsimd.memset(ones_cols, 1.0)

        def v_proj(t):
            for n in range(2):
                c = slice(512 * n, 512 * (n + 1))
                ps = psum.tile([128, 512], dt.float32, tag="ps", name="ps")
                for kd in range(8):
                    nc.tensor.matmul(
                        ps[:], tile_of("xT", kd)[:, 128 * t:128 * (t + 1)],
                        tile_of("wv", kd)[:, c], start=(kd == 0), stop=False)
                nc.tensor.matmul(ps[:], ones1[:], bv_sb[:, c],
                                 start=False, stop=True)
                # one strided cast: psum [p, 8, 64] -> v_aug cols, stride 65
                vout = v_sb[t][:].rearrange(
                    "p (h c) -> p h c", c=DH + 1)[:, 8 * n:8 * n + 8, 0:DH]
                vin = ps[:].rearrange("p (i c) -> p i c", c=DH)
                nc.vector.tensor_copy(vout, vin)

        for t in range(3):
            v_proj(t)

        # ---- attention, one head-pair (2j, 2j+1) at a time; S/exp one
        # k-tile ahead of the O accumulation so the PE never waits on exp ----
        oTs = [ctile([128, QS], dt.bfloat16, f"oTs{j}") for j in range(8)]
        dscr = ctile([1, 2 * QS], dt.float32, "dscr")
        sca = ctile([1, 2 * QS], dt.float32, "sca")
        scb = ctile([1, 2 * QS], dt.bfloat16, "scb")
        rscr = ctile([1, 2 * QS], dt.float32, "rscr")

        es_tiles = {}  # (j, kt) -> es tile

        def s_stage(j, kt):
            kc = slice(128 * kt, 128 * (kt + 1))
            sp = spair.tile([128, 2, QS], dt.float32, tag="sp", name="sp")
            nc.tensor.matmul(sp[:, 0, :], kT_sb[j][0:64, kc],
                             qTp[j][0:64, :], start=True, stop=True)
            nc.tensor.matmul(sp[:, 1, :], kT_sb[j][64:128, kc],
                             qTp[j][64:128, :], start=True, stop=True)
            es = espool.tile([128, 2, QS], dt.bfloat16, tag="es", name="es")
            nc.scalar.activation(es[:], sp[:], AF.Exp,
                                 bias=kb_c(kt), scale=0.125)
            es_tiles[(j, kt)] = es

        def o_stage(j, kt, oA, oB):
            hA, hB = 2 * j, 2 * j + 1
            es = es_tiles.pop((j, kt))
            nc.tensor.matmul(oA[0:65, :], v_sb[kt][:, 65 * hA:65 * hA + 65],
                             es[:, 0, :], start=(kt == 0), stop=(kt == 7))
            nc.tensor.matmul(oB[0:65, :], v_sb[kt][:, 65 * hB:65 * hB + 65],
                             es[:, 1, :], start=(kt == 0), stop=(kt == 7))

        def o_alloc():
            oA = psum.tile([128, QS], dt.float32, tag="ps", name="ps")
            oB = psum.tile([128, QS], dt.float32, tag="ps", name="ps")
            return oA, oB

        # flat software-pipelined stream: S/exp stages run LOOKAHEAD stages
        # ahead of the O accumulation (even across pair boundaries) so the
        # ScalarE exp pipeline never drains — it is the attention-phase
        # bottleneck and can start during the V projection
        LOOKAHEAD = 8
        stages = [(j, kt) for j in range(8) for kt in range(8)]
        s_cursor = [0]

        def advance_s(upto):
            while s_cursor[0] < min(upto, 64):
                s_stage(*stages[s_cursor[0]])
                s_cursor[0] += 1

        fps_early = [None, None]
        cur = o_alloc()
        advance_s(1)
        for j in range(8):
            oA, oB = cur
            for kt in range(1, 8):
                advance_s(8 * j + kt + LOOKAHEAD)
                o_stage(j, kt - 1, oA, oB)
                if j == 0 and kt + 2 < 8:
                    v_proj(kt + 2)
            if j < 7:
                cur = o_alloc()
            advance_s(8 * j + 8 + LOOKAHEAD)
            o_stage(j, 7, oA, oB)

            # row 64 = denominator. Free oA/oB quickly (copy to packed bf16
            # oTu) so the next pair's O accumulation gets PSUM banks while
            # the scale chain runs (DVE); K-proj matmuls are traced before
            # the sr matmuls so the in-order PE queue has filler work while
            # the reciprocal chain completes.
            nc.vector.tensor_copy(dscr[0:1, 0:QS], oA[64:65, :])
            nc.vector.tensor_copy(dscr[0:1, QS:2 * QS], oB[64:65, :])
            oTu = srpool.tile([128, QS], dt.bfloat16, tag="oTu", name="oTu")
            nc.vector.tensor_copy(oTu[0:64, :], oA[0:64, :])
            nc.vector.tensor_copy(oTu[64:128, :], oB[0:64, :])
            nc.vector.reciprocal_approx_accurate(out=sca[:], in_=dscr[:],
                                                 scratch=rscr[:])
            nc.vector.tensor_copy(scb[:], sca[:])
            # PE filler while the DVE reciprocal chain runs (the sr matmuls
            # below sit in the in-order PE queue behind it): k_proj for a
            # later pair, or for the last two pairs a partial accumulation
            # (j=0..5) of the first two output-projection tiles.
            if j + 2 < 8:
                k_proj(j + 2)
            else:
                fe = psum.tile([128, 512], dt.float32, tag="ps", name="ps")
                fc = slice(512 * (j - 6), 512 * (j - 5))
                for jj in range(6):
                    nc.tensor.matmul(fe[:], oTs[jj][:, 0:128],
                                     tile_of("wo", jj)[:, fc],
                                     start=(jj == 0), stop=False)
                fps_early[j - 6] = fe
                if j == 7:
                    # a third early tile fits PSUM here (oA/oB released by
                    # the oTu copies above): (qt1, n0) through j=6. A 4th
                    # would deadlock: sr below needs the last free slot.
                    fe2 = psum.tile([128, 512], dt.float32, tag="ps",
                                    name="ps")
                    for jj in range(7):
                        nc.tensor.matmul(fe2[:], oTs[jj][:, 128:256],
                                         tile_of("wo", jj)[:, 0:512],
                                         start=(jj == 0), stop=False)
                    fps_early.append(fe2)
                    # oTs[6] is ready now: extend the held qt0 tiles to j=6
                    nc.tensor.matmul(fps_early[0][:], oTs[6][:, 0:128],
                                     tile_of("wo", 6)[:, 0:512],
                                     start=False, stop=False)
                    nc.tensor.matmul(fps_early[1][:], oTs[6][:, 0:128],
                                     tile_of("wo", 6)[:, 512:1024],
                                     start=False, stop=False)
            sr = psum.tile([128, QS], dt.float32, tag="ps", name="ps")
            nc.tensor.matmul(sr[0:64, :], ones64[:], scb[:, 0:QS],
                             start=True, stop=True)
            nc.tensor.matmul(sr[64:128, :], ones64[:], scb[:, QS:2 * QS],
                             start=True, stop=True, tile_position=(0, 64))
            nc.vector.tensor_mul(oTs[j][:], oTu[:], sr[:])

        # ---- output projection: out[q, d] = (O^T.T @ Wo) * q_mask + bo ----
        for qt in (1, 2, 3, 0):  # pair-7-dependent resumes last
            qr = slice(128 * qt, 128 * (qt + 1))
            for n in range(2):
                c = slice(512 * n, 512 * (n + 1))
                if qt == 0 and fps_early[n] is not None:
                    ps = fps_early[n]
                    j0 = 7
                elif qt == 1 and n == 0 and len(fps_early) > 2:
                    ps = fps_early[2]
                    j0 = 7
                else:
                    ps = psum.tile([128, 512], dt.float32, tag="ps", name="ps")
                    j0 = 0
                for j in range(j0, 8):
                    nc.tensor.matmul(ps[:], oTs[j][:, qr],
                                     tile_of("wo", j)[:, c],
                                     start=(j == 0), stop=(j == 7))
                ot = opool.tile([128, 512], dt.float32, tag="osb", name="osb")
                nc.vector.scalar_tensor_tensor(
                    ot[:], ps[:], qm_c(qt), bo_rep[:, c],
                    op0=ALU.mult, op1=ALU.add)
                nc.sync.dma_start(aps["out"][qr, c], ot[:])


def get_nc():
    if "nc" not in _NC_CACHE:
        _NC_CACHE["nc"] = _build_nc()
    return _NC_CACHE["nc"]


def make_in_maps(q, x, q_mask, k_mask, Wq, bq, Wk, bk, Wv, bv, Wo, bo):
    """Host-side shard/layout prep. Returns in_maps for cores 0..7."""
    wq_b = Wq.astype(BF16)
    wk_b = Wk.astype(BF16)
    wv_b = Wv.astype(BF16)
    wo_b = Wo.astype(BF16)
    bv_r = bv.astype(BF16).reshape(1, D)
    bo_r = bo.astype(BF16).reshape(1, D)
    bq_p = bq.astype(np.float32).reshape(8, 128).T
    bk_p = bk.astype(np.float32).reshape(8, 128).T

    in_maps = []
    for c in range(NCORES):
        b, qh = c // 2, c % 2
        qs = slice(QS * qh, QS * (qh + 1))
        kbias = np.where(k_mask[b] != 0, 0.0, NEG).astype(np.float32)
        consts = np.empty((128, 28), np.float32)
        consts[:, 0:8] = bq_p
        consts[:, 8:16] = bk_p
        consts[:, 16:24] = kbias.reshape(8, 128).T
        consts[:, 24:28] = q_mask[b, qs].astype(np.float32).reshape(4, 128).T
        in_maps.append({
            "qT": np.ascontiguousarray(q[b, qs, :].T).astype(BF16),
            "xT": np.ascontiguousarray(x[b].T).astype(BF16),
            "Wq": wq_b, "Wk": wk_b, "Wv": wv_b, "Wo": wo_b,
            "consts": np.ascontiguousarray(consts),
            "bvr": bv_r, "bor": bo_r,
        })
    return in_maps


def kernel(q, x, q_mask, k_mask, Wq, bq, Wk, bk, Wv, bv, Wo, bo):
    from concourse import bass_utils

    q = np.asarray(q, np.float32)
    x = np.asarray(x, np.float32)
    q_mask = np.asarray(q_mask)
    k_mask = np.asarray(k_mask)

    nc = get_nc()
    in_maps = make_in_maps(q, x, q_mask, k_mask, Wq, bq, Wk, bk, Wv, bv, Wo, bo)
    res = bass_utils.run_bass_kernel_spmd(nc, in_maps, core_ids=list(range(NCORES)))

    out = np.empty((B, LQ, D), np.float32)
    for c in range(NCORES):
        b, qh = c // 2, c % 2
        out[b, QS * qh:QS * (qh + 1), :] = res.results[c]["out"]
    return out



# revision 31
# speedup vs baseline: 1.1296x; 1.1296x over previous
"""Multi-head attention (B=4, L=1024, D=1024, H=16) on 8 TRN2 NeuronCores.

Sharding: pure data-parallel over (batch, query-half) — core c handles batch
c//2, query rows [512*(c%2), 512*(c%2+1)). No collectives; the host
concatenates the 8 output slices.

v2 rewrite (from trace analysis of the v1 kernel):
  * DMA loads are phase-serialized (qT+Wq -> xT+Wk -> Wv+Wo) via gate ops +
    manual deps so the first projection starts at ~9us instead of ~33us
    (v1 let all 16 DMA rings run concurrently -> everything landed at ~30us).
  * Keep-alive matmuls warm the PE HAM clock-gate during the load window.
  * Attention starts right after Q-proj + K-proj(0) (~25us, v1: ~66us);
    remaining K/V projections are PE-queue filler inside the pair stream.
  * O matmuls are column-tiled (two M=64 heads at tile_position (0,0)/(0,64))
    instead of two serial M=65 — the ones-column denominator is replaced by
    a DVE+GPSIMD exp-sum accumulation and two col-tiled M=1 matmuls.
  * Each pair's scale (sr) matmuls are deferred ~2us of PE work into the next
    pair so the reciprocal chain never blocks the in-order PE queue (v1 lost
    ~4.9us/pair to this), and the reciprocal is the single-pass approx_fast.
  * Scale multiply reads O-PSUM and sr-PSUM directly (no oTu copy).
  * bv bias is folded into the V cast (kills 16 K=1 bias matmuls).

Layouts (all transposed, no transposes anywhere):
  Q^T[vd, q] = Wq(lhsT) @ qT(rhs)  (+bq per-partition)
  K^T[vd, k] = Wk(lhsT) @ xT(rhs)  (+bk per-partition)
  V  [k, vd] = xT(lhsT) @ Wv(rhs)  (+bv via DVE add of bv_rep)
  S^T[k, 2, q] = K^T_h(lhsT, K=64) @ Q^T_h  per head PAIR, row-tiled
  es = exp(S^T/8 + kmask_bias)   (ScalarE, PSUM->SBUF bf16)
  acc = sum_kt es                (DVE half + GPSIMD half, bf16)
  den = ones^T @ acc             (two col-tiled M=1 matmuls)
  O^T[128, q] = [V_h0|V_h1](lhsT, M=64 each, col-tiled) @ es
  oTs = O^T * (1/den broadcast)  (DVE, both operands PSUM)
  out[q, d] = (oTs.T @ Wo) * q_mask + bo  (DVE STT epilogue)
"""

import os

os.environ.setdefault("MYCRO_LOCAL_CACHE", "1")

import numpy as np
import ml_dtypes

BF16 = ml_dtypes.bfloat16

B, LQ, LK = 4, 1024, 1024
D = 1024  # QD = KD = VD
H, DH = 16, 64
QS = 512  # queries per core
NCORES = 8
NEG = -1e4  # additive key-mask bias (exp(-1e4) == 0)

_NC_CACHE = {}


def _build_nc():
    import concourse.bacc as bacc
    import concourse.mybir as mybir
    import concourse.tile as tile

    dt = mybir.dt

    nc = bacc.Bacc(
        "TRN2",
        debug=False,
        target_bir_lowering=False,
        num_devices=NCORES,
    )

    def din(name, shape, dtype):
        return nc.dram_tensor(name, shape, dtype, kind="ExternalInput").ap()

    aps = {
        "qT": din("qT", [D, QS], dt.bfloat16),
        "xT": din("xT", [D, LK], dt.bfloat16),
        "Wq": din("Wq", [D, D], dt.bfloat16),
        "Wk": din("Wk", [D, D], dt.bfloat16),
        "Wv": din("Wv", [D, D], dt.bfloat16),
        "Wo": din("Wo", [D, D], dt.bfloat16),
        # packed per-partition constants: cols 0-7 bq, 8-15 bk, 16-23 kbias,
        # 24-27 q_mask (by query tile)
        "consts": din("consts", [128, 28], dt.float32),
        "out": nc.dram_tensor("out", [QS, D], dt.float32,
                              kind="ExternalOutput").ap(),
    }
    if os.environ.get("KDEBUG"):
        aps["dbg_acc"] = nc.dram_tensor(
            "dbg_acc", [8, 128, 2, QS], dt.float32, kind="ExternalOutput").ap()
        aps["dbg_scb"] = nc.dram_tensor(
            "dbg_scb", [8, 2 * QS], dt.float32, kind="ExternalOutput").ap()
        aps["dbg_ots"] = nc.dram_tensor(
            "dbg_ots", [8, 128, QS], dt.float32, kind="ExternalOutput").ap()

    with tile.TileContext(nc) as tc:
        _body(tc, dt, mybir, aps)

    nc.compile()
    return nc


def _body(tc, dt, mybir, aps):
    from contextlib import ExitStack
    from concourse.tile import add_dep_helper

    ALU = mybir.AluOpType
    AF = mybir.ActivationFunctionType
    nc = tc.nc
    with ExitStack() as ctx:
        const = ctx.enter_context(tc.tile_pool(name="const", bufs=1))
        dbgpool = (ctx.enter_context(tc.tile_pool(name="dbg", bufs=2))
                   if "dbg_acc" in aps else None)
        espool = ctx.enter_context(tc.tile_pool(name="es", bufs=10))
        accpool = ctx.enter_context(tc.tile_pool(name="acc", bufs=3))
        scpool = ctx.enter_context(tc.tile_pool(name="sc", bufs=2))
        spair = ctx.enter_context(tc.tile_pool(name="sp", bufs=2, space="PSUM"))
        oppool = ctx.enter_context(tc.tile_pool(name="op", bufs=2, space="PSUM"))
        gpool = ctx.enter_context(tc.tile_pool(name="g", bufs=2, space="PSUM"))
        opool = ctx.enter_context(tc.tile_pool(name="osb", bufs=3))

        def ctile(shape, dtype, tag):
            return const.tile(shape, dtype, tag=tag, name=tag)

        def gtile():
            return gpool.tile([128, 512], dt.float32, tag="g", name="g")

        # ---- tiny class-0 DMAs ----
        # (bv/bo are folded in EXACTLY on the host: softmax rows sum to 1, so
        # out += q_mask * (bv @ Wo) + bo after the gather)
        cst = ctile([128, 28], dt.float32, "cst")
        nc.sync.dma_start(cst[:], aps["consts"][:, :])

        bq_c = lambda j: cst[:, j:j + 1]
        bk_c = lambda j: cst[:, 8 + j:9 + j]
        kb_c = lambda kt: cst[:, 16 + kt:17 + kt]
        qm_c = lambda qt: cst[:, 24 + qt:25 + qt]

        # ones / junk (vector engine is idle early)
        ones1 = ctile([1, 128], dt.bfloat16, "ones1")
        nc.vector.memset(ones1[:], 1.0)
        ones512 = ctile([1, 512], dt.bfloat16, "ones512")
        nc.vector.memset(ones512[:], 1.0)
        ones64 = ctile([1, 64], dt.bfloat16, "ones64")
        nc.vector.memset(ones64[:], 1.0)
        onescol = ctile([128, 1], dt.bfloat16, "onescol")
        nc.vector.memset(onescol[:], 1.0)
        gjunk = ctile([1, 16], dt.float32, "gjunk")
        ejunk = ctile([1, 16], dt.float32, "ejunk")
        nc.vector.memset(ejunk[:], 1.0)
        # pull the exp ACT table load off the critical path
        nc.scalar.activation(ejunk[:], ejunk[:], AF.Exp, bias=0.0, scale=1.0)

        # ---- keep-alive matmuls: hold the PE HAM clock-gate warm while the
        # input DMAs stream in (PE would otherwise idle >3.4us and re-throttle
        # to 1.2 GHz right as the projections start) ----
        ka = gtile()
        for _ in range(30):
            nc.tensor.matmul(ka[:], ones1[:], ones512[:], start=True, stop=True)

        # ---- input loads, phase-serialized: A: qT+Wq, B: xT+Wk, C: Wv+Wo.
        # Without gating all DMA rings run concurrently and share bandwidth,
        # so *everything* (including Wq) lands at ~30us. ----
        engs = [nc.sync, nc.scalar, nc.gpsimd]
        ei = [0]

        def load8(name, dram, nfree):
            tl = ctile([128, 8, nfree], dt.bfloat16, name)
            view = dram.rearrange("(t p) n -> p t n", p=128)
            insts = []
            for t in range(8):
                insts.append(engs[ei[0] % 3].dma_start(tl[:, t, :], view[:, t, :]))
                ei[0] += 1
            return tl, insts

        qT_sb, _ = load8("qT", aps["qT"], QS)
        wq_sb, _ = load8("wq", aps["Wq"], D)
        gA = nc.vector.tensor_tensor(
            gjunk[0:1, 0:8], qT_sb[0:1, :, 0:1], wq_sb[0:1, :, 0:1], ALU.add)
        xT_sb, bi1 = load8("xT", aps["xT"], LK)
        wk_sb, bi2 = load8("wk", aps["Wk"], D)
        for inst in bi1 + bi2:
            add_dep_helper(inst.ins, gA.ins, reason="dma class B waits on A")
        gB = nc.vector.tensor_tensor(
            gjunk[0:1, 8:16], xT_sb[0:1, :, 0:1], wk_sb[0:1, :, 0:1], ALU.add)
        wv_sb, ci1 = load8("wv", aps["Wv"], D)
        wo_sb, ci2 = load8("wo", aps["Wo"], D)
        for inst in ci1 + ci2:
            add_dep_helper(inst.ins, gB.ins, reason="dma class C waits on B")

        # ---- projections ----
        qTp = [ctile([128, QS], dt.bfloat16, f"qTp{j}") for j in range(8)]

        def q_proj(j):
            ps = gtile()
            for kt in range(8):
                nc.tensor.matmul(ps[:], wq_sb[:, kt, 128 * j:128 * (j + 1)],
                                 qT_sb[:, kt, :], start=(kt == 0), stop=(kt == 7))
            nc.vector.tensor_scalar_add(qTp[j][:], ps[:], bq_c(j))

        kT_sb = [ctile([128, LK], dt.bfloat16, f"kT{j}") for j in range(8)]

        def k_proj_half(j, n):
            c = slice(512 * n, 512 * (n + 1))
            ps = gtile()
            for kt in range(8):
                nc.tensor.matmul(ps[:], wk_sb[:, kt, 128 * j:128 * (j + 1)],
                                 xT_sb[:, kt, c], start=(kt == 0), stop=(kt == 7))
            nc.vector.tensor_scalar_add(kT_sb[j][:, c], ps[:], bk_c(j))

        v_sb = [ctile([128, D], dt.bfloat16, f"v{t}") for t in range(8)]

        def v_proj(t, n):
            c = slice(512 * n, 512 * (n + 1))
            ps = gtile()
            for kd in range(8):
                nc.tensor.matmul(ps[:], xT_sb[:, kd, 128 * t:128 * (t + 1)],
                                 wv_sb[:, kd, c], start=(kd == 0), stop=(kd == 7))
            nc.vector.tensor_copy(v_sb[t][:, c], ps[:])

        # ---- attention ----
        oTs = [ctile([128, QS], dt.bfloat16, f"oTs{j}") for j in range(8)]
        es_tiles = {}
        acc_last = {}
        scb_of = {}

        def s_stage(j, kt):
            kc = slice(128 * kt, 128 * (kt + 1))
            sp = spair.tile([128, 2, QS], dt.float32, tag="sp", name="sp")
            nc.tensor.matmul(sp[:, 0, :], kT_sb[j][0:64, kc],
                             qTp[j][0:64, :], start=True, stop=True)
            nc.tensor.matmul(sp[:, 1, :], kT_sb[j][64:128, kc],
                             qTp[j][64:128, :], start=True, stop=True)
            es = espool.tile([128, 2, QS], dt.bfloat16, tag="es", name="es")
            nc.scalar.activation(es[:], sp[:], AF.Exp,
                                 bias=kb_c(kt), scale=0.125)
            es_tiles[(j, kt)] = es
            if kt == 0:
                acc_last[j] = es
            else:
                a = accpool.tile([128, 2, QS], dt.bfloat16, tag="acc", name="acc")
                prev = acc_last[j]
                nc.vector.tensor_add(a[:, 0, :], prev[:, 0, :], es[:, 0, :])
                nc.gpsimd.tensor_add(a[:, 1, :], prev[:, 1, :], es[:, 1, :])
                acc_last[j] = a

        def o_stage(j, kt, oP):
            hA, hB = 2 * j, 2 * j + 1
            es = es_tiles.pop((j, kt))
            # skip_group_check: the sim's PSUM zero-region bookkeeping mixes
            # up partition-base offsets (>0) with intra-partition addresses;
            # the two groups live on disjoint partitions so HW per-element
            # has_written handles them fine.
            nc.tensor.matmul(oP[0:64, :], v_sb[kt][:, 64 * hA:64 * hA + 64],
                             es[:, 0, :], start=(kt == 0), stop=(kt == 7),
                             tile_position=(0, 0), skip_group_check=True)
            nc.tensor.matmul(oP[64:128, :], v_sb[kt][:, 64 * hB:64 * hB + 64],
                             es[:, 1, :], start=(kt == 0), stop=(kt == 7),
                             tile_position=(0, 64), skip_group_check=True)

        def den_stage(j):
            a = acc_last.pop(j)
            if "dbg_acc" in aps:
                dbg = dbgpool.tile([128, 2, QS], dt.float32, tag="da", name="da")
                nc.vector.tensor_copy(dbg[:], a[:])
                nc.scalar.dma_start(aps["dbg_acc"][j, :, :, :], dbg[:])
            # reciprocal_approx_fast (custom DVE op) drops the partition base
            # of its input AP — it must read partition 0. So the two
            # denominators go to partition 0 of two separate PSUM tiles.
            dpA = gtile()
            nc.tensor.matmul(dpA[0:1, :], onescol[:], a[:, 0, :],
                             start=True, stop=True)
            dpB = gtile()
            nc.tensor.matmul(dpB[0:1, :], onescol[:], a[:, 1, :],
                             start=True, stop=True)
            sca = scpool.tile([1, 2 * QS], dt.float32, tag="sca", name="sca")
            scb = scpool.tile([1, 2 * QS], dt.bfloat16, tag="scb", name="scb")
            nc.vector.reciprocal_approx_fast(out=sca[:, 0:QS], in_=dpA[0:1, :])
            nc.vector.reciprocal_approx_fast(out=sca[:, QS:2 * QS],
                                             in_=dpB[0:1, :])
            nc.scalar.copy(scb[:], sca[:])  # cast on ScalarE: DVE is loaded
            if "dbg_scb" in aps:
                nc.scalar.dma_start(aps["dbg_scb"][j:j + 1, :], sca[:])
            scb_of[j] = scb

        def sr_stage(j, oP):
            scb = scb_of.pop(j)
            sr = gtile()
            nc.tensor.matmul(sr[0:64, :], ones64[:], scb[:, 0:QS],
                             start=True, stop=True)
            nc.tensor.matmul(sr[64:128, :], ones64[:], scb[:, QS:2 * QS],
                             start=True, stop=True, tile_position=(0, 64),
                             skip_group_check=True)
            # DVE can read at most one PSUM operand: stage sr to SBUF first
            srs = scpool.tile([128, QS], dt.bfloat16, tag="srs", name="srs")
            nc.vector.tensor_copy(srs[:], sr[:])
            nc.vector.tensor_mul(oTs[j][:], oP[:], srs[:])
            if "dbg_ots" in aps:
                dbg = dbgpool.tile([128, QS], dt.float32, tag="do", name="do")
                nc.vector.tensor_copy(dbg[:], oP[:])
                nc.scalar.dma_start(aps["dbg_ots"][j, :, :], dbg[:])

        # out-projection early work: partial j-chains staged to SBUF during
        # pairs 6/7 (transient PSUM use) + one PSUM-resident chain in a freed
        # O-accumulator buffer, so the post-attention drain is short.
        stage_sbuf = {}

        def stage_partial(qt, n, upto):
            c = slice(512 * n, 512 * (n + 1))
            qr = slice(128 * qt, 128 * (qt + 1))
            ps = gtile()
            for jj in range(upto):
                nc.tensor.matmul(ps[:], oTs[jj][:, qr], wo_sb[:, jj, c],
                                 start=(jj == 0), stop=(jj == upto - 1))
            st = ctile([128, 512], dt.float32, f"stg{qt}{n}")
            nc.vector.tensor_scalar_mul(st[:], ps[:], qm_c(qt))  # pre-scale
            stage_sbuf[(qt, n)] = (st, upto)

        fps_state = {}

        def fps_emit(count):
            # qt0/n0 chain in a recycled op-pool buffer (free after sr(6))
            if "ap" not in fps_state:
                fps_state["ap"] = oppool.tile([128, QS], dt.float32,
                                              tag="op", name="op")
                fps_state["next_j"] = 0
            for _ in range(count):
                j = fps_state["next_j"]
                nc.tensor.matmul(fps_state["ap"][:], oTs[j][:, 0:128],
                                 wo_sb[:, j, 0:512], start=(j == 0),
                                 stop=(j == 7))
                fps_state["next_j"] += 1

        def out_epilogue(ps_ap, qt, n):
            c = slice(512 * n, 512 * (n + 1))
            qr = slice(128 * qt, 128 * (qt + 1))
            ot = opool.tile([128, 512], dt.float32, tag="osb", name="osb")
            nc.vector.tensor_scalar_mul(ot[:], ps_ap, qm_c(qt))
            nc.sync.dma_start(aps["out"][qr, c], ot[:])

        def drain_staged(qt, n):
            st, upto = stage_sbuf[(qt, n)]
            c = slice(512 * n, 512 * (n + 1))
            qr = slice(128 * qt, 128 * (qt + 1))
            ps = gtile()
            for jj in range(upto, 8):
                nc.tensor.matmul(ps[:], oTs[jj][:, qr], wo_sb[:, jj, c],
                                 start=(jj == upto), stop=(jj == 7))
            ot = opool.tile([128, 512], dt.float32, tag="osb", name="osb")
            nc.vector.scalar_tensor_tensor(
                ot[:], ps[:], qm_c(qt), st[:],
                op0=ALU.mult, op1=ALU.add)
            nc.sync.dma_start(aps["out"][qr, c], ot[:])

        # ---- schedule ----
        for j in range(8):
            q_proj(j)
        k_proj_half(0, 0)
        k_proj_half(0, 1)
        k_proj_half(1, 0)
        k_proj_half(1, 1)
        v_proj(0, 0)
        v_proj(1, 0)

        # per-(pair, kt) PE filler emissions
        fillers = {}
        for kt in range(6):  # V n=0 tiles just-in-time for pair 0's O stages
            fillers[(0, kt)] = [lambda t=kt + 2: v_proj(t, 0)]
        fillers[(0, 6)] = [lambda: k_proj_half(2, 0)]
        fillers[(0, 7)] = [lambda: k_proj_half(2, 1)]
        fillers[(1, 0)] = [lambda: v_proj(0, 1)]
        fillers[(1, 1)] = [lambda: v_proj(1, 1)]
        fillers[(1, 2)] = [lambda: v_proj(2, 1)]
        fillers[(1, 4)] = [lambda: k_proj_half(3, 0)]
        fillers[(1, 5)] = [lambda: k_proj_half(3, 1)]
        fillers[(2, 0)] = [lambda: v_proj(3, 1)]
        fillers[(2, 1)] = [lambda: v_proj(4, 1)]
        fillers[(2, 2)] = [lambda: v_proj(5, 1)]
        fillers[(2, 4)] = [lambda: k_proj_half(4, 0)]
        fillers[(2, 5)] = [lambda: k_proj_half(4, 1)]
        fillers[(3, 0)] = [lambda: v_proj(6, 1)]
        fillers[(3, 1)] = [lambda: v_proj(7, 1)]
        fillers[(3, 4)] = [lambda: k_proj_half(5, 0)]
        fillers[(3, 5)] = [lambda: k_proj_half(5, 1)]
        fillers[(4, 4)] = [lambda: k_proj_half(6, 0)]
        fillers[(4, 5)] = [lambda: k_proj_half(6, 1)]
        fillers[(5, 4)] = [lambda: k_proj_half(7, 0)]
        fillers[(5, 5)] = [lambda: k_proj_half(7, 1)]
        # early out-proj work in pairs 6/7 (oTs[j] ready after sr_stage(j)
        # at pair j+1 kt2)
        fillers[(6, 3)] = [lambda: stage_partial(0, 1, 6)]
        fillers[(6, 5)] = [lambda: stage_partial(1, 0, 6)]
        fillers[(7, 3)] = [lambda: stage_partial(1, 1, 7)]
        fillers[(7, 4)] = [lambda: fps_emit(4)]
        fillers[(7, 5)] = [lambda: stage_partial(2, 1, 7)]
        fillers[(7, 6)] = [lambda: fps_emit(3)]

        LOOK = 5
        s_cursor = [0]

        def advance_s(upto):
            while s_cursor[0] < min(upto, 64):
                jj, kk = divmod(s_cursor[0], 8)
                s_stage(jj, kk)
                s_cursor[0] += 1

        advance_s(LOOK)
        oP_prev = None
        for j in range(8):
            oP = oppool.tile([128, QS], dt.float32, tag="op", name="op")
            for kt in range(8):
                advance_s(8 * j + kt + 1 + LOOK)
                o_stage(j, kt, oP)
                for f in fillers.get((j, kt), []):
                    f()
                if kt == 2 and j > 0:
                    sr_stage(j - 1, oP_prev)
            den_stage(j)
            oP_prev = oP

        # ---- drain ----
        # den(7) already emitted; cover its recip window with the j<7 matmuls
        # of a full out tile, then scale pair 7 and finish everything.
        ps_qt2n0 = gtile()
        for jj in range(7):
            nc.tensor.matmul(ps_qt2n0[:], oTs[jj][:, 256:384],
                             wo_sb[:, jj, 0:512], start=(jj == 0), stop=False)
        sr_stage(7, oP_prev)
        fps_emit(1)  # j=7 for qt0/n0
        out_epilogue(fps_state["ap"][:], 0, 0)
        nc.tensor.matmul(ps_qt2n0[:], oTs[7][:, 256:384], wo_sb[:, 7, 0:512],
                         start=False, stop=True)
        out_epilogue(ps_qt2n0[:], 2, 0)
        drain_staged(1, 1)
        drain_staged(2, 1)
        drain_staged(0, 1)
        drain_staged(1, 0)
        for qt, n in ((3, 0), (3, 1)):
            c = slice(512 * n, 512 * (n + 1))
            ps = gtile()
            for jj in range(8):
                nc.tensor.matmul(ps[:], oTs[jj][:, 128 * qt:128 * (qt + 1)],
                                 wo_sb[:, jj, c], start=(jj == 0),
                                 stop=(jj == 7))
            out_epilogue(ps[:], qt, n)


def get_nc():
    if "nc" not in _NC_CACHE:
        _NC_CACHE["nc"] = _build_nc()
    return _NC_CACHE["nc"]


def make_in_maps(q, x, q_mask, k_mask, Wq, bq, Wk, bk, Wv, bv, Wo, bo):
    """Host-side shard/layout prep. Returns in_maps for cores 0..7."""
    wq_b = Wq.astype(BF16)
    wk_b = Wk.astype(BF16)
    wv_b = Wv.astype(BF16)
    wo_b = Wo.astype(BF16)
    bq_p = bq.astype(np.float32).reshape(8, 128).T
    bk_p = bk.astype(np.float32).reshape(8, 128).T

    in_maps = []
    for c in range(NCORES):
        b, qh = c // 2, c % 2
        qs = slice(QS * qh, QS * (qh + 1))
        kbias = np.where(k_mask[b] != 0, 0.0, NEG).astype(np.float32)
        consts = np.empty((128, 28), np.float32)
        consts[:, 0:8] = bq_p
        consts[:, 8:16] = bk_p
        consts[:, 16:24] = kbias.reshape(8, 128).T
        consts[:, 24:28] = q_mask[b, qs].astype(np.float32).reshape(4, 128).T
        in_maps.append({
            "qT": np.ascontiguousarray(q[b, qs, :].T).astype(BF16),
            "xT": np.ascontiguousarray(x[b].T).astype(BF16),
            "Wq": wq_b, "Wk": wk_b, "Wv": wv_b, "Wo": wo_b,
            "consts": np.ascontiguousarray(consts),
        })
    return in_maps


def kernel(q, x, q_mask, k_mask, Wq, bq, Wk, bk, Wv, bv, Wo, bo):
    from concourse import bass_utils

    q = np.asarray(q, np.float32)
    x = np.asarray(x, np.float32)
    q_mask = np.asarray(q_mask)
    k_mask = np.asarray(k_mask)

    nc = get_nc()
    in_maps = make_in_maps(q, x, q_mask, k_mask, Wq, bq, Wk, bk, Wv, bv, Wo, bo)
    res = bass_utils.run_bass_kernel_spmd(nc, in_maps, core_ids=list(range(NCORES)))

    out = np.empty((B, LQ, D), np.float32)
    for c in range(NCORES):
        b, qh = c // 2, c % 2
        out[b, QS * qh:QS * (qh + 1), :] = res.results[c]["out"]
    # exact host-side bias fold: attn rows sum to 1 pre-q_mask, so
    # out = (attn@V)@Wo  on device  and  +q_mask*(bv@Wo) + bo  here.
    bvwo = np.asarray(bv, np.float32) @ np.asarray(Wo, np.float32)
    bo_f = np.asarray(bo, np.float32)
    if np.any(bvwo) or np.any(bo_f):
        out += (q_mask.astype(np.float32)[:, :, None] * bvwo[None, None, :]
                + bo_f[None, None, :])
    return out
